# revision 12
# baseline (speedup 1.0000x reference)
"""FASTMultiHeadAttention (fastmax p=2, causal) Bass/Tile kernel for TRN2.

Per core: H heads (b*h pairs), each [n, d] with d=32, chunked causal scan
at C=128 using moment tensors:
  s(q,k) = A0 + A1*(q.k)*sc2 + A2*((q.k)*sc2)^2,  sc2 = 1/sqrt(d)
        = 0.5*(sc2*(q.k) + 1)^2 + 0.5         (A0=1, A1=1, A2=0.5)
Intra-chunk: exact scores + causal mask.  Inter-chunk: moments
  M1 = [S1 | Z1] = k^T [v|1]   (d x 33)
  S0row = 1^T [v|1]            (1 x 33)
  M2 = kk^T [v|1]              (768 x 33), kk = 3-block outer features:
     AA: k_a*k_b (a,b in 0..16), BB (16..32), AB (a in A, b in B; x2 on q side)
q side features carry A2*sc2^2 (and x2 for AB).
"""

import numpy as np
import concourse.bass as bass
import concourse.mybir as mybir
import concourse.tile as tile
from concourse.masks import make_identity, make_upper_triangular

F32 = mybir.dt.float32
BF16 = mybir.dt.bfloat16
MULT = mybir.AluOpType.mult
ADD = mybir.AluOpType.add
AF = mybir.ActivationFunctionType


def build_kernel(H=4, n=4096, d=32, C=128, gp_builds=True,
                 do_tp=True, do_scores=True, do_feats=True, do_qqT=True,
                 do_low=True, do_high=True, dbg_nostt=False,
                 dbg_norecip=False, dbg_nonormtt=False, dbg_notp4=False,
                 dbg_noscores=False, dbg_nointra=False, dbg_nosq=False, dbg_h1=False,
                 dbg_noA0=False, dbg_noA1=False, sim_safe=False, hw_sem=True):
    assert d == 32 and C == 128
    NCH = n // C
    dh = d // 2  # 16
    FEAT = 768  # 3-block symmetric basis: AA(256) + BB(256) + AB(256, x2 on q)
    NS = FEAT // 128  # 6 slices
    sc2 = 1.0 / float(np.sqrt(d))  # scale^2
    A1s = sc2
    A2s = 0.5 * sc2 * sc2

    nc = bass.Bass()
    q_ext = nc.declare_dram_parameter("q", [H, n, d], F32, isOutput=False)
    k_ext = nc.declare_dram_parameter("k", [H, n, d], F32, isOutput=False)
    v_ext = nc.declare_dram_parameter("v", [H, n, d], F32, isOutput=False)
    out_ext = nc.declare_dram_parameter("out", [H, n, d], F32, isOutput=True)

    HD = H * d  # 128 when H=4
    HC = H * C  # 512
    H33 = H * 33

    from contextlib import ExitStack
    with ExitStack() as ctx:
        e = ctx.enter_context
        # constants
        ident_b = e(nc.sbuf_tensor([128, 128], BF16))
        trilhalf_b = e(nc.sbuf_tensor([128, 128], BF16))  # 0.5 where i >= j
        ones_b = e(nc.sbuf_tensor([128, 128], BF16))      # all ones
        # per-chunk staging (double buffered by python alternation)
        qn_f = e(nc.sbuf_tensor([128, 3, HD], F32))
        kn_f = e(nc.sbuf_tensor([128, 3, HD], F32))
        vn_f = e(nc.sbuf_tensor([128, 3, HD], F32))
        qn_b = e(nc.sbuf_tensor([128, 3, HD], BF16))
        qA2_b = e(nc.sbuf_tensor([128, 3, HD], BF16))      # A2s * q
        qA4_b = e(nc.sbuf_tensor([128, 3, H * dh], BF16))  # 2*A2s * q[:, 0:16]
        kn_b = e(nc.sbuf_tensor([128, 3, H * 33], BF16))  # [k|1] per head
        vn1_b = e(nc.sbuf_tensor([128, 3, H33], BF16))   # [v|1] per head
        qT_b = e(nc.sbuf_tensor([33, 3, H * C], BF16))   # [d;1] x (h, pos)
        kT_b = e(nc.sbuf_tensor([32, 3, H * C], BF16))
        y_f = e(nc.sbuf_tensor([128, 4, HC], F32))       # (1+sc2*x)^2
        sT_b = e(nc.sbuf_tensor([128, 4, HC], BF16))     # masked scores bf16
        kkF_b = e(nc.sbuf_tensor([128, 3, H * FEAT], BF16))
        qqN_b = e(nc.sbuf_tensor([128, 3, H * FEAT], BF16))
        qqT_b = e(nc.sbuf_tensor([128, 3, H * FEAT], BF16))
        m1s0_snap = e(nc.sbuf_tensor([33, 132], BF16))   # [S1|Z1;S0|Z0] snaps
        a1vec_f = e(nc.sbuf_tensor([33, 1], F32))        # per-row scale
        m2_snap = e(nc.sbuf_tensor([128, H, NS * 33], BF16))  # per head
        recip_f = e(nc.sbuf_tensor([128, 3, H], F32))
        o_f = e(nc.sbuf_tensor([128, 3, HD], F32))
        # psum: 8 banks
        # m2all: 3 banks = m2 heads at 256*h [0:1024) + m1 at [1024:1156)
        m2all_ps = e(nc.psum_tensor([128, 1536], F32))
        nm_ps = e(nc.psum_tensor([128, 2, 512], F32))  # num accum (2 banks)
        sT_ps = e(nc.psum_tensor([128, 2, HC], F32))  # scores (dbl, 2 banks)
        tp_ps = e(nc.psum_tensor([128, 2, 512], BF16))  # qT/kT staging (1 bank)
        with tile.TileContext(nc) as tc:
            # ---- init constants ----
            make_identity(nc, ident_b[:, :])
            make_upper_triangular(nc, trilhalf_b[:, :], val=0.5, diag=True)
            nc.vector.memset(ones_b[:, :], 1.0)
            nc.vector.memset(a1vec_f[0:32, 0:1], A1s)
            nc.vector.memset(a1vec_f[32:33, 0:1], 1.0)
            m1_ps = m2all_ps[0:33, 1024:1156]
            nc.vector.memset(m2all_ps[:, :], 0.0)
            nc.vector.memset(m2all_ps[:, :], 0.0)
            for b in range(3):
                nc.vector.memset(vn1_b[:, b, :], 1.0)  # ones cols pre-set
                nc.vector.memset(kn_b[:, b, :], 1.0)
                nc.vector.memset(qT_b[32:33, b, :], 1.0)  # ones row

            blkctr = [0]

            def m2ps_h(h):  # [128, NS*33] region of head h
                return m2all_ps[:, 256 * h:256 * h + NS * 33]

            def emit_loads(cl):
                bl = cl % 3
                rl = cl * C
                nc.sync.dma_start(
                    out=qn_f[:, bl, :].rearrange("p (h x) -> p h x", h=H),
                    in_=q_ext[:, rl:rl + C, :].transpose([1, 0, 2]))
                nc.sync.dma_start(
                    out=kn_f[:, bl, :].rearrange("p (h x) -> p h x", h=H),
                    in_=k_ext[:, rl:rl + C, :].transpose([1, 0, 2]))
                nc.sync.dma_start(
                    out=vn_f[:, bl, :].rearrange("p (h x) -> p h x", h=H),
                    in_=v_ext[:, rl:rl + C, :].transpose([1, 0, 2]))

            for c in range(NCH):
                b = c % 3
                r0 = c * C
                # ---- loads: prefetch one chunk ahead ----
                if c == 0:
                    emit_loads(0)
                if c + 1 < NCH:
                    emit_loads(c + 1)
                # ---- casts (ACT) ----
                nc.scalar.copy(qn_b[:, b, :], qn_f[:, b, :])
                nc.scalar.mul(qA2_b[:, b, :], qn_f[:, b, :], A2s)
                nc.scalar.mul(
                    qA4_b[:, b, :].rearrange("p (h x) -> p h x", h=H),
                    qn_f[:, b, :].rearrange("p (h x) -> p h x", h=H)[:, :, 0:dh],
                    2.0 * A2s)
                nc.scalar.copy(
                    kn_b[:, b, :].rearrange("p (h x) -> p h x", h=H)[:, :, 0:d],
                    kn_f[:, b, :].rearrange("p (h x) -> p h x", h=H))
                # v -> [v|1] bf16 (ones cols pre-set)
                nc.scalar.copy(
                    vn1_b[:, b, :].rearrange("p (h x) -> p h x", h=H)[:, :, 0:d],
                    vn_f[:, b, :].rearrange("p (h x) -> p h x", h=H))
                # ---- qT/kT via PE transpose ----
                if not do_tp:
                    nc.scalar.copy(o_f[:, b, :], vn_f[:, b, :])
                    for h in range(H):
                        nc.sync.dma_start(out=out_ext[h, r0:r0 + C, :],
                                          in_=o_f[:, b, h * d:(h + 1) * d])
                    continue
                for h in range(H):
                    nc.tensor.matmul(
                        tp_ps[0:32, 0, h * C:(h + 1) * C],
                        lhsT=qn_b[:, b, h * d:(h + 1) * d], rhs=ident_b[:, :],
                        is_transpose=True, start=True, stop=True)
                    nc.tensor.matmul(
                        tp_ps[0:32, 1, h * C:(h + 1) * C],
                        lhsT=kn_b[:, b, h * 33:h * 33 + d], rhs=ident_b[:, :],
                        is_transpose=True, start=True, stop=True)
                nc.scalar.copy(qT_b[0:32, b, :], tp_ps[0:32, 0, 0:H * C])
                nc.scalar.copy(kT_b[:, b, :], tp_ps[0:32, 1, 0:H * C])

                # ---- scores: sT[j,i] per head (row-packed) ----
                if not do_scores:
                    nc.scalar.copy(o_f[:, b, :], vn_f[:, b, :])
                    for h in range(H):
                        nc.sync.dma_start(out=out_ext[h, r0:r0 + C, :],
                                          in_=o_f[:, b, h * d:(h + 1) * d])
                    continue
                for h in range((1 if dbg_h1 else H) if not dbg_noscores else 0):
                    nc.tensor.matmul(
                        sT_ps[:, c % 2, h * C:(h + 1) * C],
                        lhsT=kT_b[:, b, h * C:(h + 1) * C],
                        rhs=qT_b[0:32, b, h * C:(h + 1) * C],
                        start=(h == 0 or not hw_sem), stop=True)
                # poly: y = (sc2*x + 1)^2 ; s = (y+1)*trilhalf -> bf16
                if dbg_noscores or dbg_nosq:
                    nc.vector.memset(y_f[:, b, :], 1.0)
                else:
                    nc.scalar.activation(y_f[:, b, :], sT_ps[:, c % 2, :], AF.Square,
                                         bias=1.0, scale=sc2)
                if dbg_nostt:
                    nc.scalar.copy(sT_b[:, b, :], y_f[:, b, :])
                else:
                    nc.vector.scalar_tensor_tensor(
                        out=sT_b[:, b, :].rearrange("p (h x) -> p h x", h=H),
                        in0=y_f[:, b, :].rearrange("p (h x) -> p h x", h=H),
                        scalar=1.0,
                        in1=trilhalf_b[:, :].unsqueeze(1).broadcast_to(
                            [128, H, C]),
                        op0=ADD, op1=MULT)

                # ---- moment updates for PREVIOUS chunk (pipelined) ----
                # kk/kn/vn of chunk c-1 are long ready -> PE never stalls on
                # the builds, and next-chunk work can proceed ahead of them.
                if c >= 1 and (do_low or do_high):
                    cm1 = c - 1
                    bp = cm1 % 3
                    st = (cm1 == 0)
                    sp = (cm1 == NCH - 2) or sim_safe
                    for h in range(H):
                        vb1 = vn1_b[:, bp, h * 33:(h + 1) * 33]
                        nc.tensor.matmul(
                            m1_ps[:, h * 33:(h + 1) * 33],
                            lhsT=kn_b[:, bp, h * 33:(h + 1) * 33], rhs=vb1,
                            start=(st and not hw_sem), stop=sp)
                        reg = m2ps_h(h)
                        for g in range(NS if (do_high and do_feats) else 0):
                            nc.tensor.matmul(
                                reg[:, g * 33:(g + 1) * 33],
                                lhsT=kkF_b[:, bp, (h * NS + g) * 128:
                                           (h * NS + g + 1) * 128],
                                rhs=vb1,
                                start=(st and not hw_sem), stop=sp)

                # ---- snapshots (capture moments through chunk c-1) ----
                if c > 0 and (do_low or do_high):
                    nc.scalar.activation(m1s0_snap[0:33, 0:132], m1_ps,
                                         AF.Copy, scale=a1vec_f[0:33, 0:1])
                    if do_high and do_feats:
                        nc.scalar.copy(
                            m2_snap[:, :, :],
                            m2all_ps[:, 0:1024].rearrange(
                                "p (h x) -> p h x", h=4)[:, :, 0:NS * 33])

                # ---- feature builds: AA | BB | AB blocks, plain TT ----
                # qq carries A2s (AA/BB) or 2*A2s (AB) via pre-scaled copies.
                # qq on Vector (emitted first, feeds DMA transposes); kk on
                # GpSimd.  Two-head merged DMA transposes (cost ~fixed/call).
                def build3(eng, dst, r_, t_):
                    for g in range(3):
                        eng.tensor_tensor(
                            out=dst[:, g * 256:(g + 1) * 256].rearrange(
                                "p (a x) -> p a x", a=dh),
                            in0=r_[g].unsqueeze(2).broadcast_to([128, dh, dh]),
                            in1=t_[g].unsqueeze(1).broadcast_to([128, dh, dh]),
                            op=MULT)

                for h in range(H if do_feats else 0):
                    qq_h = qqN_b[:, b, h * FEAT:(h + 1) * FEAT]
                    qa2 = qA2_b[:, b, h * d:(h + 1) * d]
                    qa4 = qA4_b[:, b, h * dh:(h + 1) * dh]
                    qpl = qn_b[:, b, h * d:(h + 1) * d]
                    build3(nc.vector, qq_h,
                           (qa2[:, 0:dh], qa2[:, dh:d], qa4[:, 0:dh]),
                           (qpl[:, 0:dh], qpl[:, dh:d], qpl[:, dh:d]))
                    if do_qqT and h % 2 == 1:
                        h0 = h - 1
                        nc.sync.dma_start_transpose(
                            qqT_b[:, b, h0 * FEAT:(h + 1) * FEAT].rearrange(
                                "p (g x) -> p g x", g=2 * NS),
                            qqN_b[:, b, h0 * FEAT:(h + 1) * FEAT])
                for h in range(H if do_feats else 0):
                    kk_h = kkF_b[:, b, h * FEAT:(h + 1) * FEAT]
                    kpl = kn_b[:, b, h * 33:h * 33 + d]
                    keng = nc.gpsimd if (gp_builds and h < 3) else nc.vector
                    build3(keng, kk_h,
                           (kpl[:, 0:dh], kpl[:, dh:d], kpl[:, 0:dh]),
                           (kpl[:, 0:dh], kpl[:, dh:d], kpl[:, dh:d]))

                # ---- intra mask (after builds in V queue) ----
                if dbg_nostt:
                    nc.scalar.copy(sT_b[:, b, :], y_f[:, b, :])
                else:
                    nc.vector.scalar_tensor_tensor(
                        out=sT_b[:, b, :].rearrange("p (h x) -> p h x", h=H),
                        in0=y_f[:, b, :].rearrange("p (h x) -> p h x", h=H),
                        scalar=1.0,
                        in1=trilhalf_b[:, :].unsqueeze(1).broadcast_to(
                            [128, H, C]),
                        op0=ADD, op1=MULT)

                # ---- num accumulation (low -> high -> intra order) ----
                num_ps = nm_ps[:, c % 2, 0:H33]
                ops = []  # (h, lhsT, rhs)
                if c > 0 and do_low:
                    for h in range(H):
                        ops.append((h, qT_b[:, b, h * C:(h + 1) * C],
                                    m1s0_snap[0:33, h * 33:(h + 1) * 33]))
                if c > 0 and do_low and do_high and do_qqT and do_feats:
                    for h in range(H):
                        msn = m2_snap[:, h, :]
                        for g in range(NS):
                            ops.append((h, qqT_b[:, b, (h * NS + g) * 128:
                                                 (h * NS + g + 1) * 128],
                                        msn[:, g * 33:(g + 1) * 33]))
                if not dbg_nointra:
                    for h in range(H):
                        ops.append((h, sT_b[:, b, h * C:(h + 1) * C],
                                    vn1_b[:, b, h * 33:(h + 1) * 33]))
                first_i = {}
                last_i = {}
                for i, (h, _, _) in enumerate(ops):
                    first_i.setdefault(h, i)
                    last_i[h] = i
                for i, (h, lhsT, rhs) in enumerate(ops):
                    st_ = (i == 0) if hw_sem else (i == first_i[h])
                    nc.tensor.matmul(
                        num_ps[:, h * 33:(h + 1) * 33], lhsT=lhsT, rhs=rhs,
                        start=st_, stop=(i == last_i[h]))

                # ---- normalization + store for PREVIOUS chunk ----
                def emit_out(co):
                    bo = co % 3
                    nro = nm_ps[:, co % 2, 0:H33].rearrange(
                        "p (h x) -> p h x", h=H)
                    if dbg_norecip:
                        nc.vector.memset(recip_f[:, bo, :], 1.0)
                    else:
                        nc.vector.reciprocal(recip_f[:, bo, :], nro[:, :, 32])
                    if dbg_nonormtt:
                        nc.scalar.copy(
                            o_f[:, bo, :].rearrange("p (h x) -> p h x", h=H),
                            nro[:, :, 0:32])
                    else:
                        nc.vector.tensor_tensor(
                            out=o_f[:, bo, :].rearrange("p (h x) -> p h x",
                                                        h=H),
                            in0=nro[:, :, 0:32],
                            in1=recip_f[:, bo, :].unsqueeze(2).broadcast_to(
                                [128, H, d]),
                            op=MULT)
                    nc.sync.dma_start(
                        out=out_ext[:, co * C:co * C + C, :].transpose(
                            [1, 0, 2]),
                        in_=o_f[:, bo, :].rearrange("p (h x) -> p h x", h=H))

                if c >= 1:
                    emit_out(c - 1)
            emit_out(NCH - 1)
    return nc


def ref_fastmax(q, k, v):
    """numpy reference, q/k/v [H, n, d], returns [H, n, d]."""
    H, n, d = q.shape
    sc = 1.0 / d ** 0.25
    qs, ks = q * sc, k * sc
    out = np.zeros_like(q)
    for h in range(H):
        x = qs[h] @ ks[h].T  # [n, n]
        s = 1.0 + x + 0.5 * x * x
        s *= np.tril(np.ones((n, n), np.float32))
        num = s @ v[h]
        den = s.sum(-1, keepdims=True)
        out[h] = num / den
    return out


# ============================================================================
# walrus in this container accepts at most 1 sync-wait per instruction;
# split longer on_wait lists into same-engine NoOp prefixes.
# ============================================================================
MAX_WAITS = 1


def split_waits(nc):
    n_split = 0
    for f in nc.m.functions:
        for bb in f.blocks:
            insts = bb.instructions
            out = []
            for inst in insts:
                si = inst.sync_info
                waits = list(si.on_wait) if si and si.on_wait else []
                if len(waits) > MAX_WAITS:
                    n_split += 1
                    head, rest = waits[:-MAX_WAITS], waits[-MAX_WAITS:]
                    chunks = [head[i:i + MAX_WAITS]
                              for i in range(0, len(head), MAX_WAITS)]
                    for j, chunk in enumerate(chunks):
                        out.append(mybir.InstNoOp(
                            name=f"{inst.name}_wsplit{j}",
                            engine=inst.engine,
                            ins=[], outs=[],
                            sync_info=mybir.SyncInfo(on_wait=chunk,
                                                     on_update=[]),
                            bass_nofuse=True,
                        ))
                    si.on_wait = rest
                out.append(inst)
            insts.clear()
            insts.extend(out)
    return n_split


# ============================================================================
# SPMD wrapper: full inputs -> shard batch*heads over 8 cores -> full output
# ============================================================================
import os as _os
_os.environ.setdefault("NEURON_RT_RESET_CORES", "1")
_CACHE = {}


def _get_nc(H, n):
    key = (H, n)
    if key not in _CACHE:
        nc = build_kernel(H=H, n=n, gp_builds=True, hw_sem=True)
        split_waits(nc)
        _CACHE[key] = nc
    return _CACHE[key]


def kernel(q, k, v, drop_noise=None, mask=1, p=2):
    from concourse.bass_utils import run_bass_kernel_spmd

    assert int(p) == 2 and int(mask) == 1, "kernel specialized for mask=1, p=2"
    q = np.ascontiguousarray(np.asarray(q, dtype=np.float32))
    k = np.ascontiguousarray(np.asarray(k, dtype=np.float32))
    v = np.ascontiguousarray(np.asarray(v, dtype=np.float32))
    bsz, nh, n, d = q.shape
    B = bsz * nh
    ncores = 8
    hp = B // ncores
    qf = q.reshape(B, n, d)
    kf = k.reshape(B, n, d)
    vf = v.reshape(B, n, d)
    in_maps = [
        {"q": np.ascontiguousarray(qf[i * hp:(i + 1) * hp]),
         "k": np.ascontiguousarray(kf[i * hp:(i + 1) * hp]),
         "v": np.ascontiguousarray(vf[i * hp:(i + 1) * hp])}
        for i in range(ncores)
    ]
    nc = _get_nc(hp, n)
    res = None
    for attempt in range(3):
        try:
            res = run_bass_kernel_spmd(nc, in_maps,
                                       core_ids=list(range(ncores)))
            break
        except Exception:
            if attempt == 2:
                raise
    outs = [np.asarray(res.results[i]["out"]).reshape(hp, n, d)
            for i in range(ncores)]
    return np.concatenate(outs, axis=0).reshape(bsz, nh, n, d)



# revision 17
# speedup vs baseline: 1.0088x; 1.0088x over previous
"""FASTMultiHeadAttention (fastmax p=2, causal) Bass/Tile kernel for TRN2.

Per core: H heads (b*h pairs), each [n, d] with d=32, chunked causal scan
at C=128 using moment tensors:
  s(q,k) = A0 + A1*(q.k)*sc2 + A2*((q.k)*sc2)^2,  sc2 = 1/sqrt(d)
        = 0.5*(sc2*(q.k) + 1)^2 + 0.5         (A0=1, A1=1, A2=0.5)
Intra-chunk: exact scores + causal mask.  Inter-chunk: moments
  M1 = [S1 | Z1] = k^T [v|1]   (d x 33)
  S0row = 1^T [v|1]            (1 x 33)
  M2 = kk^T [v|1]              (768 x 33), kk = 3-block outer features:
     AA: k_a*k_b (a,b in 0..16), BB (16..32), AB (a in A, b in B; x2 on q side)
q side features carry A2*sc2^2 (and x2 for AB).
"""

import numpy as np
import concourse.bass as bass
import concourse.mybir as mybir
import concourse.tile as tile
from concourse.masks import make_identity, make_upper_triangular

F32 = mybir.dt.float32
BF16 = mybir.dt.bfloat16
MULT = mybir.AluOpType.mult
ADD = mybir.AluOpType.add
AF = mybir.ActivationFunctionType


def build_kernel(H=4, n=4096, d=32, C=128, gp_builds=True,
                 do_tp=True, do_scores=True, do_feats=True, do_qqT=True,
                 do_low=True, do_high=True, dbg_nostt=False,
                 dbg_norecip=False, dbg_nonormtt=False, dbg_notp4=False,
                 dbg_noscores=False, dbg_nointra=False, dbg_nosq=False, dbg_h1=False,
                 dbg_noA0=False, dbg_noA1=False, sim_safe=False, hw_sem=True):
    assert d == 32 and C == 128
    NCH = n // C
    dh = d // 2  # 16
    FEAT = 768  # 3-block symmetric basis: AA(256) + BB(256) + AB(256, x2 on q)
    NS = FEAT // 128  # 6 slices
    sc2 = 1.0 / float(np.sqrt(d))  # scale^2
    A1s = sc2
    A2s = 0.5 * sc2 * sc2

    nc = bass.Bass()
    q_ext = nc.declare_dram_parameter("q", [H, n, d], F32, isOutput=False)
    k_ext = nc.declare_dram_parameter("k", [H, n, d], F32, isOutput=False)
    v_ext = nc.declare_dram_parameter("v", [H, n, d], F32, isOutput=False)
    out_ext = nc.declare_dram_parameter("out", [H, n, d], F32, isOutput=True)

    HD = H * d  # 128 when H=4
    HC = H * C  # 512
    H33 = H * 33

    from contextlib import ExitStack
    with ExitStack() as ctx:
        e = ctx.enter_context
        # constants
        ident_b = e(nc.sbuf_tensor([128, 128], BF16))
        trilhalf_b = e(nc.sbuf_tensor([128, 128], BF16))  # 0.5 where i >= j
        ones_b = e(nc.sbuf_tensor([128, 128], BF16))      # all ones
        # per-chunk staging (double buffered by python alternation)
        qn_f = e(nc.sbuf_tensor([128, 3, HD], F32))
        kn_f = e(nc.sbuf_tensor([128, 3, HD], F32))
        vn_f = e(nc.sbuf_tensor([128, 3, HD], F32))
        qn_b = e(nc.sbuf_tensor([128, 3, HD], BF16))
        qA2_b = e(nc.sbuf_tensor([128, 3, HD], BF16))      # A2s * q
        qA4_b = e(nc.sbuf_tensor([128, 3, H * dh], BF16))  # 2*A2s * q[:, 0:16]
        kn_b = e(nc.sbuf_tensor([128, 3, H * 33], BF16))  # [k|1] per head
        vn1_b = e(nc.sbuf_tensor([128, 3, H33], BF16))   # [v|1] per head
        qT_b = e(nc.sbuf_tensor([33, 3, H * C], BF16))   # [d;1] x (h, pos)
        kT_b = e(nc.sbuf_tensor([32, 3, H * C], BF16))
        y_f = e(nc.sbuf_tensor([128, 4, HC], F32))       # (1+sc2*x)^2
        sT_b = e(nc.sbuf_tensor([128, 4, HC], BF16))     # masked scores bf16
        kkF_l = [e(nc.sbuf_tensor(f"kkF{i}", [128, H * FEAT], BF16))
                 for i in range(3)]
        qqN_b = e(nc.sbuf_tensor([128, 3, H * FEAT], BF16))
        qqT_b = e(nc.sbuf_tensor([128, 3, H * FEAT], BF16))
        m1s0_snap = e(nc.sbuf_tensor([33, 132], BF16))   # [S1|Z1;S0|Z0] snaps
        a1vec_f = e(nc.sbuf_tensor([33, 1], F32))        # per-row scale
        m2_snap = e(nc.sbuf_tensor([128, H, NS * 33], BF16))  # per head
        recip_f = e(nc.sbuf_tensor([128, 3, H], F32))
        o_f = e(nc.sbuf_tensor([128, 3, HD], F32))
        # psum: 8 banks
        # m2all: 3 banks = m2 heads at 256*h [0:1024) + m1 at [1024:1156)
        m2all_ps = e(nc.psum_tensor([128, 1536], F32))
        nm_ps = e(nc.psum_tensor([128, 2, 512], F32))  # num accum (2 banks)
        sT_ps = e(nc.psum_tensor([128, 2, HC], F32))  # scores (dbl, 2 banks)
        tp_ps = e(nc.psum_tensor([128, 2, 512], BF16))  # qT/kT staging (1 bank)
        with tile.TileContext(nc) as tc:
            # ---- init constants ----
            make_identity(nc, ident_b[:, :])
            make_upper_triangular(nc, trilhalf_b[:, :], val=0.5, diag=True)
            nc.vector.memset(ones_b[:, :], 1.0)
            nc.vector.memset(a1vec_f[0:32, 0:1], A1s)
            nc.vector.memset(a1vec_f[32:33, 0:1], 1.0)
            m1_ps = m2all_ps[0:33, 1024:1156]
            nc.vector.memset(m2all_ps[:, :], 0.0)
            nc.vector.memset(m2all_ps[:, :], 0.0)
            for b in range(3):
                nc.vector.memset(vn1_b[:, b, :], 1.0)  # ones cols pre-set
                nc.vector.memset(kn_b[:, b, :], 1.0)
                nc.vector.memset(qT_b[32:33, b, :], 1.0)  # ones row

            blkctr = [0]

            def m2ps_h(h):  # [128, NS*33] region of head h
                return m2all_ps[:, 256 * h:256 * h + NS * 33]

            def emit_loads(cl):
                bl = cl % 3
                rl = cl * C
                nc.sync.dma_start(
                    out=qn_f[:, bl, :].rearrange("p (h x) -> p h x", h=H),
                    in_=q_ext[:, rl:rl + C, :].transpose([1, 0, 2]))
                nc.sync.dma_start(
                    out=kn_f[:, bl, :].rearrange("p (h x) -> p h x", h=H),
                    in_=k_ext[:, rl:rl + C, :].transpose([1, 0, 2]))
                nc.sync.dma_start(
                    out=vn_f[:, bl, :].rearrange("p (h x) -> p h x", h=H),
                    in_=v_ext[:, rl:rl + C, :].transpose([1, 0, 2]))

            for c in range(NCH):
                b = c % 3
                r0 = c * C
                # ---- loads: prefetch one chunk ahead ----
                if c == 0:
                    emit_loads(0)
                if c + 1 < NCH:
                    emit_loads(c + 1)
                # ---- casts (ACT) ----
                nc.scalar.copy(qn_b[:, b, :], qn_f[:, b, :])
                nc.scalar.mul(qA2_b[:, b, :], qn_f[:, b, :], A2s)
                nc.scalar.mul(
                    qA4_b[:, b, :].rearrange("p (h x) -> p h x", h=H),
                    qn_f[:, b, :].rearrange("p (h x) -> p h x", h=H)[:, :, 0:dh],
                    2.0 * A2s)
                nc.scalar.copy(
                    kn_b[:, b, :].rearrange("p (h x) -> p h x", h=H)[:, :, 0:d],
                    kn_f[:, b, :].rearrange("p (h x) -> p h x", h=H))
                # v -> [v|1] bf16 (ones cols pre-set)
                nc.scalar.copy(
                    vn1_b[:, b, :].rearrange("p (h x) -> p h x", h=H)[:, :, 0:d],
                    vn_f[:, b, :].rearrange("p (h x) -> p h x", h=H))
                # ---- qT/kT via PE transpose ----
                if not do_tp:
                    nc.scalar.copy(o_f[:, b, :], vn_f[:, b, :])
                    for h in range(H):
                        nc.sync.dma_start(out=out_ext[h, r0:r0 + C, :],
                                          in_=o_f[:, b, h * d:(h + 1) * d])
                    continue
                for h in range(H):
                    nc.tensor.matmul(
                        tp_ps[0:32, 0, h * C:(h + 1) * C],
                        lhsT=qn_b[:, b, h * d:(h + 1) * d], rhs=ident_b[:, :],
                        is_transpose=True, start=True, stop=True)
                    nc.tensor.matmul(
                        tp_ps[0:32, 1, h * C:(h + 1) * C],
                        lhsT=kn_b[:, b, h * 33:h * 33 + d], rhs=ident_b[:, :],
                        is_transpose=True, start=True, stop=True)
                nc.scalar.copy(qT_b[0:32, b, :], tp_ps[0:32, 0, 0:H * C])
                nc.scalar.copy(kT_b[:, b, :], tp_ps[0:32, 1, 0:H * C])

                # ---- scores: sT[j,i] per head (row-packed) ----
                if not do_scores:
                    nc.scalar.copy(o_f[:, b, :], vn_f[:, b, :])
                    for h in range(H):
                        nc.sync.dma_start(out=out_ext[h, r0:r0 + C, :],
                                          in_=o_f[:, b, h * d:(h + 1) * d])
                    continue
                for h in range((1 if dbg_h1 else H) if not dbg_noscores else 0):
                    nc.tensor.matmul(
                        sT_ps[:, c % 2, h * C:(h + 1) * C],
                        lhsT=kT_b[:, b, h * C:(h + 1) * C],
                        rhs=qT_b[0:32, b, h * C:(h + 1) * C],
                        start=(h == 0 or not hw_sem), stop=True)
                # poly: y = (sc2*x + 1)^2 ; s = (y+1)*trilhalf -> bf16
                if dbg_noscores or dbg_nosq:
                    nc.vector.memset(y_f[:, b, :], 1.0)
                else:
                    nc.scalar.activation(y_f[:, b, :], sT_ps[:, c % 2, :], AF.Square,
                                         bias=1.0, scale=sc2)
                if dbg_nostt:
                    nc.scalar.copy(sT_b[:, b, :], y_f[:, b, :])
                else:
                    nc.vector.scalar_tensor_tensor(
                        out=sT_b[:, b, :].rearrange("p (h x) -> p h x", h=H),
                        in0=y_f[:, b, :].rearrange("p (h x) -> p h x", h=H),
                        scalar=1.0,
                        in1=trilhalf_b[:, :].unsqueeze(1).broadcast_to(
                            [128, H, C]),
                        op0=ADD, op1=MULT)

                # ---- moment updates for PREVIOUS chunk (pipelined) ----
                # kk/kn/vn of chunk c-1 are long ready -> PE never stalls on
                # the builds, and next-chunk work can proceed ahead of them.
                if c >= 1 and (do_low or do_high):
                    cm1 = c - 1
                    bp = cm1 % 3
                    st = (cm1 == 0)
                    sp = (cm1 == NCH - 2) or sim_safe
                    for h in range(H):
                        vb1 = vn1_b[:, bp, h * 33:(h + 1) * 33]
                        nc.tensor.matmul(
                            m1_ps[:, h * 33:(h + 1) * 33],
                            lhsT=kn_b[:, bp, h * 33:(h + 1) * 33], rhs=vb1,
                            start=(st and not hw_sem), stop=sp)
                        reg = m2ps_h(h)
                        for g in range(NS if (do_high and do_feats) else 0):
                            nc.tensor.matmul(
                                reg[:, g * 33:(g + 1) * 33],
                                lhsT=kkF_l[bp][:, (h * NS + g) * 128:
                                               (h * NS + g + 1) * 128],
                                rhs=vb1,
                                start=(st and not hw_sem), stop=sp)

                # ---- snapshots (capture moments through chunk c-1) ----
                if c > 0 and (do_low or do_high):
                    nc.scalar.activation(m1s0_snap[0:33, 0:132], m1_ps,
                                         AF.Copy, scale=a1vec_f[0:33, 0:1])
                    if do_high and do_feats:
                        nc.scalar.copy(
                            m2_snap[:, :, :],
                            m2all_ps[:, 0:1024].rearrange(
                                "p (h x) -> p h x", h=4)[:, :, 0:NS * 33])

                # ---- feature builds: AA | BB | AB blocks, plain TT ----
                # qq carries A2s (AA/BB) or 2*A2s (AB) via pre-scaled copies.
                # qq on Vector (emitted first, feeds DMA transposes); kk on
                # GpSimd.  Two-head merged DMA transposes (cost ~fixed/call).
                def build3(eng, dst, r_, t_):
                    for g in range(3):
                        eng.tensor_tensor(
                            out=dst[:, g * 256:(g + 1) * 256].rearrange(
                                "p (a x) -> p a x", a=dh),
                            in0=r_[g].unsqueeze(2).broadcast_to([128, dh, dh]),
                            in1=t_[g].unsqueeze(1).broadcast_to([128, dh, dh]),
                            op=MULT)

                for h in range(H if do_feats else 0):
                    qq_h = qqN_b[:, b, h * FEAT:(h + 1) * FEAT]
                    qa2 = qA2_b[:, b, h * d:(h + 1) * d]
                    qa4 = qA4_b[:, b, h * dh:(h + 1) * dh]
                    qpl = qn_b[:, b, h * d:(h + 1) * d]
                    build3(nc.vector, qq_h,
                           (qa2[:, 0:dh], qa2[:, dh:d], qa4[:, 0:dh]),
                           (qpl[:, 0:dh], qpl[:, dh:d], qpl[:, dh:d]))
                    if do_qqT and h % 2 == 1:
                        h0 = h - 1
                        nc.sync.dma_start_transpose(
                            qqT_b[:, b, h0 * FEAT:(h + 1) * FEAT].rearrange(
                                "p (g x) -> p g x", g=2 * NS),
                            qqN_b[:, b, h0 * FEAT:(h + 1) * FEAT])
                for h in range(H if do_feats else 0):
                    kk_h = kkF_l[b][:, h * FEAT:(h + 1) * FEAT]
                    kpl = kn_b[:, b, h * 33:h * 33 + d]
                    keng = nc.gpsimd if (gp_builds and h < 3) else nc.vector
                    build3(keng, kk_h,
                           (kpl[:, 0:dh], kpl[:, dh:d], kpl[:, 0:dh]),
                           (kpl[:, 0:dh], kpl[:, dh:d], kpl[:, dh:d]))

                # ---- intra mask (after builds in V queue) ----
                if dbg_nostt:
                    nc.scalar.copy(sT_b[:, b, :], y_f[:, b, :])
                else:
                    nc.vector.scalar_tensor_tensor(
                        out=sT_b[:, b, :].rearrange("p (h x) -> p h x", h=H),
                        in0=y_f[:, b, :].rearrange("p (h x) -> p h x", h=H),
                        scalar=1.0,
                        in1=trilhalf_b[:, :].unsqueeze(1).broadcast_to(
                            [128, H, C]),
                        op0=ADD, op1=MULT)

                # ---- num accumulation (low -> high -> intra order) ----
                num_ps = nm_ps[:, c % 2, 0:H33]
                ops = []  # (h, lhsT, rhs)
                if c > 0 and do_low:
                    for h in range(H):
                        ops.append((h, qT_b[:, b, h * C:(h + 1) * C],
                                    m1s0_snap[0:33, h * 33:(h + 1) * 33]))
                if c > 0 and do_low and do_high and do_qqT and do_feats:
                    for h in range(H):
                        msn = m2_snap[:, h, :]
                        for g in range(NS):
                            ops.append((h, qqT_b[:, b, (h * NS + g) * 128:
                                                 (h * NS + g + 1) * 128],
                                        msn[:, g * 33:(g + 1) * 33]))
                if not dbg_nointra:
                    for h in range(H):
                        ops.append((h, sT_b[:, b, h * C:(h + 1) * C],
                                    vn1_b[:, b, h * 33:(h + 1) * 33]))
                first_i = {}
                last_i = {}
                for i, (h, _, _) in enumerate(ops):
                    first_i.setdefault(h, i)
                    last_i[h] = i
                for i, (h, lhsT, rhs) in enumerate(ops):
                    st_ = (i == 0) if hw_sem else (i == first_i[h])
                    nc.tensor.matmul(
                        num_ps[:, h * 33:(h + 1) * 33], lhsT=lhsT, rhs=rhs,
                        start=st_, stop=(i == last_i[h]))

                # ---- normalization + store for PREVIOUS chunk ----
                def emit_out(co):
                    bo = co % 3
                    nro = nm_ps[:, co % 2, 0:H33].rearrange(
                        "p (h x) -> p h x", h=H)
                    if dbg_norecip:
                        nc.vector.memset(recip_f[:, bo, :], 1.0)
                    else:
                        nc.vector.reciprocal(recip_f[:, bo, :], nro[:, :, 32])
                    if dbg_nonormtt:
                        nc.scalar.copy(
                            o_f[:, bo, :].rearrange("p (h x) -> p h x", h=H),
                            nro[:, :, 0:32])
                    else:
                        nc.vector.tensor_tensor(
                            out=o_f[:, bo, :].rearrange("p (h x) -> p h x",
                                                        h=H),
                            in0=nro[:, :, 0:32],
                            in1=recip_f[:, bo, :].unsqueeze(2).broadcast_to(
                                [128, H, d]),
                            op=MULT)
                    nc.sync.dma_start(
                        out=out_ext[:, co * C:co * C + C, :].transpose(
                            [1, 0, 2]),
                        in_=o_f[:, bo, :].rearrange("p (h x) -> p h x", h=H))

                if c >= 1:
                    emit_out(c - 1)
            emit_out(NCH - 1)
    return nc


def ref_fastmax(q, k, v):
    """numpy reference, q/k/v [H, n, d], returns [H, n, d]."""
    H, n, d = q.shape
    sc = 1.0 / d ** 0.25
    qs, ks = q * sc, k * sc
    out = np.zeros_like(q)
    for h in range(H):
        x = qs[h] @ ks[h].T  # [n, n]
        s = 1.0 + x + 0.5 * x * x
        s *= np.tril(np.ones((n, n), np.float32))
        num = s @ v[h]
        den = s.sum(-1, keepdims=True)
        out[h] = num / den
    return out


# ============================================================================
# walrus in this container accepts at most 1 sync-wait per instruction;
# split longer on_wait lists into same-engine NoOp prefixes.
# ============================================================================
MAX_WAITS = 1


def split_waits(nc):
    n_split = 0
    for f in nc.m.functions:
        for bb in f.blocks:
            insts = bb.instructions
            out = []
            for inst in insts:
                si = inst.sync_info
                waits = list(si.on_wait) if si and si.on_wait else []
                if len(waits) > MAX_WAITS:
                    n_split += 1
                    head, rest = waits[:-MAX_WAITS], waits[-MAX_WAITS:]
                    chunks = [head[i:i + MAX_WAITS]
                              for i in range(0, len(head), MAX_WAITS)]
                    for j, chunk in enumerate(chunks):
                        out.append(mybir.InstNoOp(
                            name=f"{inst.name}_wsplit{j}",
                            engine=inst.engine,
                            ins=[], outs=[],
                            sync_info=mybir.SyncInfo(on_wait=chunk,
                                                     on_update=[]),
                            bass_nofuse=True,
                        ))
                    si.on_wait = rest
                out.append(inst)
            insts.clear()
            insts.extend(out)
    return n_split


# ============================================================================
# SPMD wrapper: full inputs -> shard batch*heads over 8 cores -> full output
# ============================================================================
import os as _os
_os.environ.setdefault("NEURON_RT_RESET_CORES", "1")
_CACHE = {}


def _get_nc(H, n):
    key = (H, n)
    if key not in _CACHE:
        nc = build_kernel(H=H, n=n, gp_builds=True, hw_sem=True)
        split_waits(nc)
        _CACHE[key] = nc
    return _CACHE[key]


def kernel(q, k, v, drop_noise=None, mask=1, p=2):
    from concourse.bass_utils import run_bass_kernel_spmd

    assert int(p) == 2 and int(mask) == 1, "kernel specialized for mask=1, p=2"
    q = np.ascontiguousarray(np.asarray(q, dtype=np.float32))
    k = np.ascontiguousarray(np.asarray(k, dtype=np.float32))
    v = np.ascontiguousarray(np.asarray(v, dtype=np.float32))
    bsz, nh, n, d = q.shape
    B = bsz * nh
    ncores = 8
    hp = B // ncores
    qf = q.reshape(B, n, d)
    kf = k.reshape(B, n, d)
    vf = v.reshape(B, n, d)
    in_maps = [
        {"q": np.ascontiguousarray(qf[i * hp:(i + 1) * hp]),
         "k": np.ascontiguousarray(kf[i * hp:(i + 1) * hp]),
         "v": np.ascontiguousarray(vf[i * hp:(i + 1) * hp])}
        for i in range(ncores)
    ]
    nc = _get_nc(hp, n)
    res = None
    for attempt in range(3):
        try:
            res = run_bass_kernel_spmd(nc, in_maps,
                                       core_ids=list(range(ncores)))
            break
        except Exception:
            if attempt == 2:
                raise
    outs = [np.asarray(res.results[i]["out"]).reshape(hp, n, d)
            for i in range(ncores)]
    return np.concatenate(outs, axis=0).reshape(bsz, nh, n, d)

